# revision 13
# baseline (speedup 1.0000x reference)
"""Trainium2 Bass kernel for DiffeoMeshDeformer.

Strategy:
- Host: affine transform of vertices, inverse affine at the end, and a
  redundant "V4" volume layout where each voxel stores its full 2x2x2
  interpolation stencil's (d,h) corners contiguously:
      V4[d,h,w] = [fs(d,h,w), fs(d+1,h,w), fs(d,h+1,w), fs(d+1,h+1,w)]
  (each entry 4 floats: 3 channels + 0 pad, with border clamping baked in).
  One contiguous 32-float read starting at voxel (d0,h0,w0) covers all 8
  trilinear corners x 3 channels.
- Device (8 cores, SPMD): per core 35000 vertices (batch b = core//4,
  quarter = core%4). 30 Euler steps; per step per point: clip, floor/frac,
  voxel index, one 128B indirect-DMA gather per point (128 points per
  instruction, one per SBUF partition), then trilinear lerps (w, h, d) on
  the vector engine, x += r.
"""

import os
import numpy as np

import concourse.bass as bass
import concourse.mybir as mybir

B = 2
N = 140000
D = H = W = 192
NUM_STEPS = int(os.environ.get("DIFFEO_STEPS", "30"))
NCORES = 8
NPC = N * B // NCORES          # 35000 points per core
P = 128
COLS = (NPC + P - 1) // P      # 274 gather columns
NPAD = P * COLS                # 35072
NVOX = D * H * W
# Two half-step groups: DVE ops must span >=~100 elems/partition — the DVE
# pipelines back-to-back dependent instructions, and ops smaller than ~40
# elems/partition read stale data from their predecessor (HW-observed).
GROUP_SIZES = [COLS // 2, COLS - COLS // 2]
ROW = 16                       # f32 elements per V4 voxel
GELEM = 32                     # f32 elements gathered per point

F32 = mybir.dt.float32
I32 = mybir.dt.int32
Alu = mybir.AluOpType


# ---------------------------------------------------------------- host helpers

def _build_v4(flow_b):
    """flow_b [3,D,H,W] f32 -> V4 flat [NVOX+2, 16] f32 (prescaled by 1/NUM_STEPS)."""
    fs = (flow_b.astype(np.float32) * np.float32(1.0 / NUM_STEPS)).transpose(1, 2, 3, 0)
    fsp = np.zeros((D, H, W, 4), np.float32)
    fsp[..., :3] = fs
    del fs
    sd = np.concatenate([fsp[1:], fsp[-1:]], axis=0)        # d+1 clamped
    sh = np.concatenate([fsp[:, 1:], fsp[:, -1:]], axis=1)  # h+1 clamped
    sdh = np.concatenate([sd[:, 1:], sd[:, -1:]], axis=1)   # d+1,h+1 clamped
    v4 = np.empty((D, H, W, 4, 4), np.float32)
    v4[:, :, :, 0] = fsp
    v4[:, :, :, 1] = sd
    v4[:, :, :, 2] = sh
    v4[:, :, :, 3] = sdh
    del fsp, sd, sh, sdh
    v4 = v4.reshape(NVOX, ROW)
    return np.concatenate([v4, np.zeros((2, ROW), np.float32)], axis=0)


def _pack_points(pts):
    """pts [NPC,3] -> [P, 3*COLS] with point n=j*P+p at [p, 3j:3j+3]."""
    arr = np.zeros((NPAD, 3), np.float32)
    arr[:NPC] = pts
    return np.ascontiguousarray(arr.reshape(COLS, P, 3).transpose(1, 0, 2)).reshape(P, 3 * COLS)


def _unpack_points(xout):
    return xout.reshape(P, COLS, 3).transpose(1, 0, 2).reshape(NPAD, 3)[:NPC]


# ------------------------------------------------------------- numpy simulator

def _simulate_core(v4, xin):
    """Bit-faithful numpy model of the device program (for logic validation)."""
    x = xin.reshape(P, COLS, 3).copy()
    volf = v4.ravel()
    for _s in range(NUM_STEPS):
        p = np.clip(x, 0.0, 191.0)
        fl = np.floor(p)
        f = p - fl
        v = (fl[..., 0] * 36864.0 + fl[..., 1] * 192.0 + fl[..., 2]).astype(np.int64)
        # gather 32 floats per point
        idx = v[..., None] * ROW + np.arange(GELEM)
        G = volf[idx]                                # [P, COLS, 32]
        g0, g1 = G[..., :16], G[..., 16:]
        A = g0 + f[..., 2:3] * (g1 - g0)             # w-lerp  -> 16
        Bv = A[..., :8] + f[..., 1:2] * (A[..., 8:] - A[..., :8])   # h-lerp -> 8
        r = Bv[..., :4] + f[..., 0:1] * (Bv[..., 4:] - Bv[..., :4])  # d-lerp -> 4
        x = x + r[..., :3]
    return x.reshape(P, COLS * 3)


# ------------------------------------------------------------- device program

def _build_program():
    ablate = os.environ.get("DIFFEO_ABLATE", "")
    nc = bass.Bass()
    v4 = nc.dram_tensor("v4", [NVOX + 2, ROW], F32, kind="ExternalInput")
    xin = nc.dram_tensor("xin", [P, 3 * COLS], F32, kind="ExternalInput")
    voff0 = None
    if ablate == "hostoff":
        voff0 = nc.dram_tensor("voff0", [P, COLS], I32, kind="ExternalInput")
    xout = nc.dram_tensor("xout", [P, 3 * COLS], F32, kind="ExternalOutput")

    ngroups = len(GROUP_SIZES)
    gstart = np.cumsum([0] + GROUP_SIZES).tolist()
    maxg = max(GROUP_SIZES)

    # semaphore count bookkeeping (python-side, static)
    dmag_cnt = [0, 0, 0, 0]
    dve_cnt = [0]

    from contextlib import ExitStack

    with ExitStack() as ctx:
        sb = lambda name, shape, dt: ctx.enter_context(nc.sbuf_tensor(name, shape, dt))
        t_x = sb("t_x", [P, 3 * COLS], F32)
        t_f = sb("t_f", [P, 3 * COLS], F32)
        t_voff = sb("t_voff", [P, COLS], I32)
        t_g = sb("t_g", [P, ngroups * maxg * GELEM], F32)
        t_p = sb("t_p", [P, 3 * maxg], F32)
        t_fl = sb("t_fl", [P, 3 * maxg], F32)
        t_m = sb("t_m", [P, 3 * maxg], F32)
        t_wi = sb("t_wi", [P, 3 * maxg], I32)
        t_a = sb("t_a", [P, maxg], F32)
        t_b = sb("t_b", [P, maxg], F32)
        t_v = sb("t_v", [P, maxg], F32)
        t_D = sb("t_D", [P, 16 * maxg], F32)
        t_A = sb("t_A", [P, 16 * maxg], F32)
        t_D2 = sb("t_D2", [P, 8 * maxg], F32)
        t_B = sb("t_B", [P, 8 * maxg], F32)
        t_D3 = sb("t_D3", [P, 4 * maxg], F32)
        t_R = sb("t_R", [P, 4 * maxg], F32)
        dmag = [ctx.enter_context(nc.semaphore(f"dmag{i}")) for i in range(4)]
        dve_sem = ctx.enter_context(nc.semaphore("dve_sem"))
        block = ctx.enter_context(nc.Block())

        # static schedules for cross-engine wait values
        # addr_done[s][g]: dve_sem value after addr phase (s,g)
        # lerp_done[s][g]: dve_sem value after lerp phase (s,g)
        # gath_done[s][g]: (q, dmag value) after all gathers of (s,g)
        addr_done = [[0] * ngroups for _ in range(NUM_STEPS)]
        lerp_done = [[0] * ngroups for _ in range(NUM_STEPS)]
        gath_done = [[0] * ngroups for _ in range(NUM_STEPS)]
        c = 0
        for s in range(NUM_STEPS):
            for g in range(ngroups):
                c += 1
                addr_done[s][g] = c
            for g in range(ngroups):
                c += 1
                lerp_done[s][g] = c
        dve_total = c
        # preload on dmag0
        pre = [32 if ablate == "hostoff" else 16, 0, 0, 0]
        cnt = list(pre)
        for s in range(NUM_STEPS):
            q = s % 4
            for g in range(ngroups):
                cnt[q] += 16 * GROUP_SIZES[g]
                gath_done[s][g] = (q, cnt[q])

        @block.gpsimd
        def _(gpsimd):
            gpsimd.dma_start(t_x[:], xin[:]).then_inc(dmag[0], 16)
            if ablate == "hostoff":
                gpsimd.dma_start(t_voff[:], voff0[:]).then_inc(dmag[0], 16)
                gpsimd.wait_ge(dmag[0], 32)
            for s in range(NUM_STEPS):
                q = s % 4
                for g in range(ngroups):
                    gpsimd.wait_ge(dve_sem, addr_done[s][g])
                    gs, gn = gstart[g], GROUP_SIZES[g]
                    for jj in range(gn):
                        col = gs + jj
                        if ablate == "nogather":
                            gpsimd.dma_start(
                                out=t_g[:, (g * maxg + jj) * GELEM:(g * maxg + jj + 1) * GELEM],
                                in_=bass.AP(v4[:].tensor, 0, [[GELEM, P], [1, GELEM]]),
                            ).then_inc(dmag[q], 16)
                        else:
                            gpsimd.indirect_dma_start(
                                out=t_g[:, (g * maxg + jj) * GELEM:(g * maxg + jj + 1) * GELEM],
                                out_offset=None,
                                in_=v4[:],
                                in_offset=bass.IndirectOffsetOnAxis(
                                    ap=t_voff[:, col:col + 1], axis=0
                                ),
                            ).then_inc(dmag[q], 16)
            gpsimd.wait_ge(dve_sem, dve_total)
            gpsimd.dma_start(xout[:], t_x[:]).then_inc(dmag[1], 16)

        @block.vector
        def _(vector):
            tt = vector.tensor_tensor
            ts = vector.tensor_scalar
            vector.wait_ge(dmag[0], 16)
            for s in range(NUM_STEPS):
                q = s % 4
                for g in range(ngroups):
                    gs, gn = gstart[g], GROUP_SIZES[g]
                    w3 = 3 * gn
                    xg = t_x[:, 3 * gs:3 * gs + w3]
                    fg = t_f[:, 3 * gs:3 * gs + w3]
                    pg = t_p[:, :w3]
                    flg = t_fl[:, :w3]
                    mg = t_m[:, :w3]
                    wig = t_wi[:, :w3]
                    # p = clip(x, 0, 191)
                    ts(out=pg, in0=xg, scalar1=191.0, scalar2=None, op0=Alu.min)
                    ts(out=pg, in0=pg, scalar1=0.0, scalar2=None, op0=Alu.max)
                    # floor via cast + correction
                    vector.tensor_copy(out=wig, in_=pg)
                    vector.tensor_copy(out=flg, in_=wig)
                    tt(out=mg, in0=flg, in1=pg, op=Alu.is_gt)
                    tt(out=flg, in0=flg, in1=mg, op=Alu.subtract)
                    tt(out=fg, in0=pg, in1=flg, op=Alu.subtract)
                    # voxel index v = d*36864 + h*192 + w
                    fl3 = flg.rearrange("p (t c) -> p t c", c=3)
                    ag = t_a[:, :gn]
                    bg = t_b[:, :gn]
                    vg = t_v[:, :gn]
                    ts(out=ag, in0=fl3[:, :, 0], scalar1=36864.0, scalar2=None, op0=Alu.mult)
                    ts(out=bg, in0=fl3[:, :, 1], scalar1=192.0, scalar2=None, op0=Alu.mult)
                    tt(out=vg, in0=ag, in1=bg, op=Alu.add)
                    tt(out=vg, in0=vg, in1=fl3[:, :, 2], op=Alu.add)
                    voff_dst = t_wi[:, :gn].bitcast(I32) if ablate == "hostoff" else t_voff[:, gs:gs + gn]
                    vector.tensor_copy(out=voff_dst, in_=vg).then_inc(dve_sem, 1)
                for g in range(ngroups):
                    gs, gn = gstart[g], GROUP_SIZES[g]
                    qq, val = gath_done[s][g]
                    vector.wait_ge(dmag[qq], val)
                    if ablate == "nolerp":
                        vector.tensor_copy(out=t_R[:, :1], in_=t_B[:, :1]).then_inc(dve_sem, 1)
                        continue
                    gb = t_g[:, g * maxg * GELEM:(g * maxg + gn) * GELEM]
                    g0 = bass.AP(gb.tensor, gb.offset, [[gb.ap[0][0], P], [GELEM, gn], [1, 16]])
                    g1 = bass.AP(gb.tensor, gb.offset + 16, [[gb.ap[0][0], P], [GELEM, gn], [1, 16]])
                    fg = t_f[:, 3 * gs:3 * gs + 3 * gn]
                    fw = bass.AP(fg.tensor, fg.offset + 2, [[fg.ap[0][0], P], [3, gn], [0, 16]])
                    fh = bass.AP(fg.tensor, fg.offset + 1, [[fg.ap[0][0], P], [3, gn], [0, 8]])
                    fd = bass.AP(fg.tensor, fg.offset + 0, [[fg.ap[0][0], P], [3, gn], [0, 4]])
                    Dg = t_D[:, :16 * gn].rearrange("p (t e) -> p t e", e=16)
                    Ag = t_A[:, :16 * gn].rearrange("p (t e) -> p t e", e=16)
                    D2g = t_D2[:, :8 * gn].rearrange("p (t e) -> p t e", e=8)
                    Bg = t_B[:, :8 * gn].rearrange("p (t e) -> p t e", e=8)
                    D3g = t_D3[:, :4 * gn].rearrange("p (t e) -> p t e", e=4)
                    Rg = t_R[:, :4 * gn].rearrange("p (t e) -> p t e", e=4)
                    A0 = t_A[:, :16 * gn].rearrange("p (t k e) -> p t k e", k=2, e=8)
                    B0 = t_B[:, :8 * gn].rearrange("p (t k e) -> p t k e", k=2, e=4)
                    # w-lerp
                    tt(out=Dg, in0=g1, in1=g0, op=Alu.subtract)
                    if ablate == "lerp0":
                        vector.tensor_copy(out=t_R[:, :1], in_=t_B[:, :1]).then_inc(dve_sem, 1)
                        continue
                    tt(out=Dg, in0=Dg, in1=fw, op=Alu.mult)
                    tt(out=Ag, in0=Dg, in1=g0, op=Alu.add)
                    if ablate == "lerp1":
                        vector.tensor_copy(out=t_R[:, :1], in_=t_B[:, :1]).then_inc(dve_sem, 1)
                        continue
                    # h-lerp
                    tt(out=D2g, in0=A0[:, :, 1], in1=A0[:, :, 0], op=Alu.subtract)
                    tt(out=D2g, in0=D2g, in1=fh, op=Alu.mult)
                    tt(out=Bg, in0=D2g, in1=A0[:, :, 0], op=Alu.add)
                    if ablate == "lerp2":
                        vector.tensor_copy(out=t_R[:, :1], in_=t_B[:, :1]).then_inc(dve_sem, 1)
                        continue
                    # d-lerp
                    tt(out=D3g, in0=B0[:, :, 1], in1=B0[:, :, 0], op=Alu.subtract)
                    tt(out=D3g, in0=D3g, in1=fd, op=Alu.mult)
                    tt(out=Rg, in0=D3g, in1=B0[:, :, 0], op=Alu.add)
                    if ablate == "lerp3":
                        vector.tensor_copy(out=t_R[:, :1], in_=t_B[:, :1]).then_inc(dve_sem, 1)
                        continue
                    # x += r (first 3 of 4 channels)
                    xg3 = t_x[:, 3 * gs:3 * gs + 3 * gn].rearrange("p (t c) -> p t c", c=3)
                    r3 = bass.AP(t_R[:].tensor, t_R[:].offset, [[t_R[:].ap[0][0], P], [4, gn], [1, 3]])
                    tt(out=xg3, in0=xg3, in1=r3, op=Alu.add).then_inc(dve_sem, 1)

    return nc


_cached = {}


def _get_program():
    if "nc" not in _cached:
        _cached["nc"] = _build_program()
    return _cached["nc"]


LAST_EXEC_TIME_NS = None


def kernel(verts, affine, flow_field):
    global LAST_EXEC_TIME_NS
    verts = np.asarray(verts, np.float32)
    affine = np.asarray(affine, np.float32)
    flow_field = np.asarray(flow_field, np.float32)

    # host: forward affine
    pred0 = np.empty((B, N, 3), np.float32)
    for b in range(B):
        A = affine[b]
        pred0[b] = verts[b] @ A[:3, :3].T + A[:3, 3]

    v4s = [_build_v4(flow_field[b]) for b in range(B)]

    in_maps = []
    per_batch = NCORES // B
    for c in range(NCORES):
        b = c // per_batch
        q = c % per_batch
        pts = pred0[b, q * NPC:(q + 1) * NPC]
        in_maps.append({"v4": v4s[b], "xin": _pack_points(pts)})

    if os.environ.get("DIFFEO_SIM") == "1":
        results = [{"xout": _simulate_core(m["v4"], m["xin"])} for m in in_maps]
    else:
        from concourse.bass_utils import run_bass_kernel_spmd

        nc = _get_program()
        trace = os.environ.get("DIFFEO_TRACE") == "1"
        res = run_bass_kernel_spmd(nc, in_maps, core_ids=list(range(NCORES)), trace=trace)
        LAST_EXEC_TIME_NS = res.exec_time_ns
        results = res.results

    aux = np.empty((B, N, 3), np.float32)
    for c in range(NCORES):
        b = c // per_batch
        q = c % per_batch
        aux[b, q * NPC:(q + 1) * NPC] = _unpack_points(results[c]["xout"])

    flow_int = np.transpose(aux - pred0, (0, 2, 1))[..., None, None]
    pred_out = np.empty((B, N, 3), np.float32)
    for b in range(B):
        iA = np.linalg.inv(affine[b])
        pred_out[b] = aux[b] @ iA[:3, :3].T + iA[:3, 3]
    return pred_out, flow_int


# revision 14
# speedup vs baseline: 1.0066x; 1.0066x over previous
"""Trainium2 Bass kernel for DiffeoMeshDeformer.

Strategy:
- Host: affine transform of vertices, inverse affine at the end, and a
  redundant "V4" volume layout where each voxel stores its full 2x2x2
  interpolation stencil's (d,h) corners contiguously:
      V4[d,h,w] = [fs(d,h,w), fs(d+1,h,w), fs(d,h+1,w), fs(d+1,h+1,w)]
  (each entry 4 floats: 3 channels + 0 pad, with border clamping baked in).
  One contiguous 32-float read starting at voxel (d0,h0,w0) covers all 8
  trilinear corners x 3 channels.
- Device (8 cores, SPMD): per core 35000 vertices (batch b = core//4,
  quarter = core%4). 30 Euler steps; per step per point: clip, floor/frac,
  voxel index, one 128B indirect-DMA gather per point (128 points per
  instruction, one per SBUF partition), then trilinear lerps (w, h, d) on
  the vector engine, x += r.
"""

import os
import numpy as np

os.environ.setdefault("NEURON_RT_RESET_CORES", "1")

import concourse.bass as bass
import concourse.mybir as mybir

B = 2
N = 140000
D = H = W = 192
NUM_STEPS = int(os.environ.get("DIFFEO_STEPS", "30"))
NCORES = 8
NPC = N * B // NCORES          # 35000 points per core
P = 128
COLS = (NPC + P - 1) // P      # 274 gather columns
NPAD = P * COLS                # 35072
NVOX = D * H * W
# Two half-step groups: DVE ops must span >=~100 elems/partition — the DVE
# pipelines back-to-back dependent instructions, and ops smaller than ~40
# elems/partition read stale data from their predecessor (HW-observed).
GROUP_SIZES = [COLS // 2, COLS - COLS // 2]
ROW = 16                       # f32 elements per V4 voxel
GELEM = 32                     # f32 elements gathered per point

F32 = mybir.dt.float32
I32 = mybir.dt.int32
Alu = mybir.AluOpType


# ---------------------------------------------------------------- host helpers

def _build_v4(flow_b):
    """flow_b [3,D,H,W] f32 -> V4 flat [NVOX+2, 16] f32 (prescaled by 1/NUM_STEPS)."""
    fs = (flow_b.astype(np.float32) * np.float32(1.0 / NUM_STEPS)).transpose(1, 2, 3, 0)
    fsp = np.zeros((D, H, W, 4), np.float32)
    fsp[..., :3] = fs
    del fs
    sd = np.concatenate([fsp[1:], fsp[-1:]], axis=0)        # d+1 clamped
    sh = np.concatenate([fsp[:, 1:], fsp[:, -1:]], axis=1)  # h+1 clamped
    sdh = np.concatenate([sd[:, 1:], sd[:, -1:]], axis=1)   # d+1,h+1 clamped
    v4 = np.empty((D, H, W, 4, 4), np.float32)
    v4[:, :, :, 0] = fsp
    v4[:, :, :, 1] = sd
    v4[:, :, :, 2] = sh
    v4[:, :, :, 3] = sdh
    del fsp, sd, sh, sdh
    v4 = v4.reshape(NVOX, ROW)
    return np.concatenate([v4, np.zeros((2, ROW), np.float32)], axis=0)


def _pack_points(pts):
    """pts [NPC,3] -> [P, 3*COLS] with point n=j*P+p at [p, 3j:3j+3]."""
    arr = np.zeros((NPAD, 3), np.float32)
    arr[:NPC] = pts
    return np.ascontiguousarray(arr.reshape(COLS, P, 3).transpose(1, 0, 2)).reshape(P, 3 * COLS)


def _unpack_points(xout):
    return xout.reshape(P, COLS, 3).transpose(1, 0, 2).reshape(NPAD, 3)[:NPC]


# ------------------------------------------------------------- numpy simulator

def _simulate_core(v4, xin):
    """Bit-faithful numpy model of the device program (for logic validation)."""
    x = xin.reshape(P, COLS, 3).copy()
    volf = v4.ravel()
    for _s in range(NUM_STEPS):
        p = np.clip(x, 0.0, 191.0)
        fl = np.floor(p)
        f = p - fl
        v = (fl[..., 0] * 36864.0 + fl[..., 1] * 192.0 + fl[..., 2]).astype(np.int64)
        # gather 32 floats per point
        idx = v[..., None] * ROW + np.arange(GELEM)
        G = volf[idx]                                # [P, COLS, 32]
        g0, g1 = G[..., :16], G[..., 16:]
        A = g0 + f[..., 2:3] * (g1 - g0)             # w-lerp  -> 16
        Bv = A[..., :8] + f[..., 1:2] * (A[..., 8:] - A[..., :8])   # h-lerp -> 8
        r = Bv[..., :4] + f[..., 0:1] * (Bv[..., 4:] - Bv[..., :4])  # d-lerp -> 4
        x = x + r[..., :3]
    return x.reshape(P, COLS * 3)


# ------------------------------------------------------------- device program

def _build_program():
    ablate = os.environ.get("DIFFEO_ABLATE", "")
    nc = bass.Bass()
    v4 = nc.dram_tensor("v4", [NVOX + 2, ROW], F32, kind="ExternalInput")
    xin = nc.dram_tensor("xin", [P, 3 * COLS], F32, kind="ExternalInput")
    voff0 = None
    if ablate == "hostoff":
        voff0 = nc.dram_tensor("voff0", [P, COLS], I32, kind="ExternalInput")
    xout = nc.dram_tensor("xout", [P, 3 * COLS], F32, kind="ExternalOutput")

    ngroups = len(GROUP_SIZES)
    gstart = np.cumsum([0] + GROUP_SIZES).tolist()
    maxg = max(GROUP_SIZES)

    # semaphore count bookkeeping (python-side, static)
    dmag_cnt = [0, 0, 0, 0]
    dve_cnt = [0]

    from contextlib import ExitStack

    with ExitStack() as ctx:
        sb = lambda name, shape, dt: ctx.enter_context(nc.sbuf_tensor(name, shape, dt))
        t_x = sb("t_x", [P, 3 * COLS], F32)
        t_f = sb("t_f", [P, 3 * COLS], F32)
        t_voff = sb("t_voff", [P, COLS], I32)
        t_g = sb("t_g", [P, ngroups * maxg * GELEM], F32)
        t_p = sb("t_p", [P, 3 * maxg], F32)
        t_fl = sb("t_fl", [P, 3 * maxg], F32)
        t_m = sb("t_m", [P, 3 * maxg], F32)
        t_wi = sb("t_wi", [P, 3 * maxg], I32)
        t_a = sb("t_a", [P, maxg], F32)
        t_b = sb("t_b", [P, maxg], F32)
        t_v = sb("t_v", [P, maxg], F32)
        t_D = sb("t_D", [P, 16 * maxg], F32)
        t_A = sb("t_A", [P, 16 * maxg], F32)
        t_D2 = sb("t_D2", [P, 8 * maxg], F32)
        t_B = sb("t_B", [P, 8 * maxg], F32)
        t_D3 = sb("t_D3", [P, 4 * maxg], F32)
        t_R = sb("t_R", [P, 4 * maxg], F32)
        dmag = [ctx.enter_context(nc.semaphore(f"dmag{i}")) for i in range(4)]
        dve_sem = ctx.enter_context(nc.semaphore("dve_sem"))
        block = ctx.enter_context(nc.Block())

        # static schedules for cross-engine wait values
        # addr_done[s][g]: dve_sem value after addr phase (s,g)
        # lerp_done[s][g]: dve_sem value after lerp phase (s,g)
        # gath_done[s][g]: (q, dmag value) after all gathers of (s,g)
        addr_done = [[0] * ngroups for _ in range(NUM_STEPS)]
        lerp_done = [[0] * ngroups for _ in range(NUM_STEPS)]
        gath_done = [[0] * ngroups for _ in range(NUM_STEPS)]
        c = 0
        for s in range(NUM_STEPS):
            for g in range(ngroups):
                c += 1
                addr_done[s][g] = c
            for g in range(ngroups):
                c += 1
                lerp_done[s][g] = c
        dve_total = c
        # preload on dmag0
        pre = [32 if ablate == "hostoff" else 16, 0, 0, 0]
        cnt = list(pre)
        for s in range(NUM_STEPS):
            q = s % 4
            for g in range(ngroups):
                cnt[q] += 16 * GROUP_SIZES[g]
                gath_done[s][g] = (q, cnt[q])

        @block.gpsimd
        def _(gpsimd):
            gpsimd.dma_start(t_x[:], xin[:]).then_inc(dmag[0], 16)
            if ablate == "hostoff":
                gpsimd.dma_start(t_voff[:], voff0[:]).then_inc(dmag[0], 16)
                gpsimd.wait_ge(dmag[0], 32)
            for s in range(NUM_STEPS):
                q = s % 4
                for g in range(ngroups):
                    gpsimd.wait_ge(dve_sem, addr_done[s][g])
                    gs, gn = gstart[g], GROUP_SIZES[g]
                    for jj in range(gn):
                        col = gs + jj
                        if ablate == "nogather":
                            gpsimd.dma_start(
                                out=t_g[:, (g * maxg + jj) * GELEM:(g * maxg + jj + 1) * GELEM],
                                in_=bass.AP(v4[:].tensor, 0, [[GELEM, P], [1, GELEM]]),
                            ).then_inc(dmag[q], 16)
                        else:
                            gpsimd.indirect_dma_start(
                                out=t_g[:, (g * maxg + jj) * GELEM:(g * maxg + jj + 1) * GELEM],
                                out_offset=None,
                                in_=v4[:],
                                in_offset=bass.IndirectOffsetOnAxis(
                                    ap=t_voff[:, col:col + 1], axis=0
                                ),
                            ).then_inc(dmag[q], 16)
            gpsimd.wait_ge(dve_sem, dve_total)
            gpsimd.dma_start(xout[:], t_x[:]).then_inc(dmag[1], 16)

        @block.vector
        def _(vector):
            tt = vector.tensor_tensor
            ts = vector.tensor_scalar
            vector.wait_ge(dmag[0], 16)
            for s in range(NUM_STEPS):
                q = s % 4
                for g in range(ngroups):
                    gs, gn = gstart[g], GROUP_SIZES[g]
                    w3 = 3 * gn
                    xg = t_x[:, 3 * gs:3 * gs + w3]
                    fg = t_f[:, 3 * gs:3 * gs + w3]
                    pg = t_p[:, :w3]
                    flg = t_fl[:, :w3]
                    mg = t_m[:, :w3]
                    wig = t_wi[:, :w3]
                    # p = clip(x, 0, 191)
                    ts(out=pg, in0=xg, scalar1=191.0, scalar2=None, op0=Alu.min)
                    ts(out=pg, in0=pg, scalar1=0.0, scalar2=None, op0=Alu.max)
                    # floor via cast + correction
                    vector.tensor_copy(out=wig, in_=pg)
                    vector.tensor_copy(out=flg, in_=wig)
                    tt(out=mg, in0=flg, in1=pg, op=Alu.is_gt)
                    tt(out=flg, in0=flg, in1=mg, op=Alu.subtract)
                    tt(out=fg, in0=pg, in1=flg, op=Alu.subtract)
                    # voxel index v = d*36864 + h*192 + w
                    fl3 = flg.rearrange("p (t c) -> p t c", c=3)
                    ag = t_a[:, :gn]
                    bg = t_b[:, :gn]
                    vg = t_v[:, :gn]
                    ts(out=ag, in0=fl3[:, :, 0], scalar1=36864.0, scalar2=None, op0=Alu.mult)
                    ts(out=bg, in0=fl3[:, :, 1], scalar1=192.0, scalar2=None, op0=Alu.mult)
                    tt(out=vg, in0=ag, in1=bg, op=Alu.add)
                    tt(out=vg, in0=vg, in1=fl3[:, :, 2], op=Alu.add)
                    voff_dst = t_wi[:, :gn].bitcast(I32) if ablate == "hostoff" else t_voff[:, gs:gs + gn]
                    vector.tensor_copy(out=voff_dst, in_=vg).then_inc(dve_sem, 1)
                for g in range(ngroups):
                    gs, gn = gstart[g], GROUP_SIZES[g]
                    qq, val = gath_done[s][g]
                    vector.wait_ge(dmag[qq], val)
                    if ablate == "nolerp":
                        vector.tensor_copy(out=t_R[:, :1], in_=t_B[:, :1]).then_inc(dve_sem, 1)
                        continue
                    gb = t_g[:, g * maxg * GELEM:(g * maxg + gn) * GELEM]
                    g0 = bass.AP(gb.tensor, gb.offset, [[gb.ap[0][0], P], [GELEM, gn], [1, 16]])
                    g1 = bass.AP(gb.tensor, gb.offset + 16, [[gb.ap[0][0], P], [GELEM, gn], [1, 16]])
                    fg = t_f[:, 3 * gs:3 * gs + 3 * gn]
                    fw = bass.AP(fg.tensor, fg.offset + 2, [[fg.ap[0][0], P], [3, gn], [0, 16]])
                    fh = bass.AP(fg.tensor, fg.offset + 1, [[fg.ap[0][0], P], [3, gn], [0, 8]])
                    fd = bass.AP(fg.tensor, fg.offset + 0, [[fg.ap[0][0], P], [3, gn], [0, 4]])
                    Dg = t_D[:, :16 * gn].rearrange("p (t e) -> p t e", e=16)
                    Ag = t_A[:, :16 * gn].rearrange("p (t e) -> p t e", e=16)
                    D2g = t_D2[:, :8 * gn].rearrange("p (t e) -> p t e", e=8)
                    Bg = t_B[:, :8 * gn].rearrange("p (t e) -> p t e", e=8)
                    D3g = t_D3[:, :4 * gn].rearrange("p (t e) -> p t e", e=4)
                    Rg = t_R[:, :4 * gn].rearrange("p (t e) -> p t e", e=4)
                    A0 = t_A[:, :16 * gn].rearrange("p (t k e) -> p t k e", k=2, e=8)
                    B0 = t_B[:, :8 * gn].rearrange("p (t k e) -> p t k e", k=2, e=4)
                    # w-lerp
                    tt(out=Dg, in0=g1, in1=g0, op=Alu.subtract)
                    if ablate == "lerp0":
                        vector.tensor_copy(out=t_R[:, :1], in_=t_B[:, :1]).then_inc(dve_sem, 1)
                        continue
                    tt(out=Dg, in0=Dg, in1=fw, op=Alu.mult)
                    tt(out=Ag, in0=Dg, in1=g0, op=Alu.add)
                    if ablate == "lerp1":
                        vector.tensor_copy(out=t_R[:, :1], in_=t_B[:, :1]).then_inc(dve_sem, 1)
                        continue
                    # h-lerp
                    tt(out=D2g, in0=A0[:, :, 1], in1=A0[:, :, 0], op=Alu.subtract)
                    tt(out=D2g, in0=D2g, in1=fh, op=Alu.mult)
                    tt(out=Bg, in0=D2g, in1=A0[:, :, 0], op=Alu.add)
                    if ablate == "lerp2":
                        vector.tensor_copy(out=t_R[:, :1], in_=t_B[:, :1]).then_inc(dve_sem, 1)
                        continue
                    # d-lerp
                    tt(out=D3g, in0=B0[:, :, 1], in1=B0[:, :, 0], op=Alu.subtract)
                    tt(out=D3g, in0=D3g, in1=fd, op=Alu.mult)
                    tt(out=Rg, in0=D3g, in1=B0[:, :, 0], op=Alu.add)
                    if ablate == "lerp3":
                        vector.tensor_copy(out=t_R[:, :1], in_=t_B[:, :1]).then_inc(dve_sem, 1)
                        continue
                    # x += r (first 3 of 4 channels)
                    xg3 = t_x[:, 3 * gs:3 * gs + 3 * gn].rearrange("p (t c) -> p t c", c=3)
                    r3 = bass.AP(t_R[:].tensor, t_R[:].offset, [[t_R[:].ap[0][0], P], [4, gn], [1, 3]])
                    tt(out=xg3, in0=xg3, in1=r3, op=Alu.add).then_inc(dve_sem, 1)

    return nc


_cached = {}


def _get_program():
    if "nc" not in _cached:
        _cached["nc"] = _build_program()
    return _cached["nc"]


LAST_EXEC_TIME_NS = None


def kernel(verts, affine, flow_field):
    global LAST_EXEC_TIME_NS
    verts = np.asarray(verts, np.float32)
    affine = np.asarray(affine, np.float32)
    flow_field = np.asarray(flow_field, np.float32)

    # host: forward affine
    pred0 = np.empty((B, N, 3), np.float32)
    for b in range(B):
        A = affine[b]
        pred0[b] = verts[b] @ A[:3, :3].T + A[:3, 3]

    v4s = [_build_v4(flow_field[b]) for b in range(B)]

    in_maps = []
    per_batch = NCORES // B
    for c in range(NCORES):
        b = c // per_batch
        q = c % per_batch
        pts = pred0[b, q * NPC:(q + 1) * NPC]
        in_maps.append({"v4": v4s[b], "xin": _pack_points(pts)})

    if os.environ.get("DIFFEO_SIM") == "1":
        results = [{"xout": _simulate_core(m["v4"], m["xin"])} for m in in_maps]
    else:
        from concourse.bass_utils import run_bass_kernel_spmd

        nc = _get_program()
        trace = os.environ.get("DIFFEO_TRACE") == "1"
        res = run_bass_kernel_spmd(nc, in_maps, core_ids=list(range(NCORES)), trace=trace)
        LAST_EXEC_TIME_NS = res.exec_time_ns
        results = res.results

    aux = np.empty((B, N, 3), np.float32)
    for c in range(NCORES):
        b = c // per_batch
        q = c % per_batch
        aux[b, q * NPC:(q + 1) * NPC] = _unpack_points(results[c]["xout"])

    flow_int = np.transpose(aux - pred0, (0, 2, 1))[..., None, None]
    pred_out = np.empty((B, N, 3), np.float32)
    for b in range(B):
        iA = np.linalg.inv(affine[b])
        pred_out[b] = aux[b] @ iA[:3, :3].T + iA[:3, 3]
    return pred_out, flow_int


# revision 18
# speedup vs baseline: 1.0454x; 1.0385x over previous
"""Trainium2 Bass kernel for DiffeoMeshDeformer.

Strategy:
- Host: affine transform of vertices, inverse affine at the end, and a
  redundant "V4" volume layout where each voxel stores its full 2x2x2
  interpolation stencil's (d,h) corners contiguously:
      V4[d,h,w] = [fs(d,h,w), fs(d+1,h,w), fs(d,h+1,w), fs(d+1,h+1,w)]
  (each entry 4 floats: 3 channels + 0 pad, with border clamping baked in).
  One contiguous 32-float read starting at voxel (d0,h0,w0) covers all 8
  trilinear corners x 3 channels.
- Device (8 cores, SPMD): per core 35000 vertices (batch b = core//4,
  quarter = core%4). 30 Euler steps; per step per point: clip, floor/frac,
  voxel index, one 128B indirect-DMA gather per point (128 points per
  instruction, one per SBUF partition), then trilinear lerps (w, h, d) on
  the vector engine, x += r.
"""

import os
import numpy as np

os.environ.setdefault("NEURON_RT_RESET_CORES", "1")

import concourse.bass as bass
import concourse.mybir as mybir

B = 2
N = 140000
D = H = W = 192
NUM_STEPS = int(os.environ.get("DIFFEO_STEPS", "30"))
NCORES = 8
NPC = N * B // NCORES          # 35000 points per core
P = 128
COLS = (NPC + P - 1) // P      # 274 gather columns
NPAD = P * COLS                # 35072
NVOX = D * H * W
# Two half-step groups: DVE ops must span >=~100 elems/partition — the DVE
# pipelines back-to-back dependent instructions, and ops smaller than ~40
# elems/partition read stale data from their predecessor (HW-observed).
GROUP_SIZES = [COLS // 2, COLS - COLS // 2]
ROW = 16                       # f32 elements per V4 voxel
GELEM = 32                     # f32 elements gathered per point

F32 = mybir.dt.float32
I32 = mybir.dt.int32
Alu = mybir.AluOpType


# ---------------------------------------------------------------- host helpers

def _build_v4(flow_b):
    """flow_b [3,D,H,W] f32 -> V4 flat [NVOX+2, 16] f32 (prescaled by 1/NUM_STEPS)."""
    fs = (flow_b.astype(np.float32) * np.float32(1.0 / NUM_STEPS)).transpose(1, 2, 3, 0)
    fsp = np.zeros((D, H, W, 4), np.float32)
    fsp[..., :3] = fs
    del fs
    sd = np.concatenate([fsp[1:], fsp[-1:]], axis=0)        # d+1 clamped
    sh = np.concatenate([fsp[:, 1:], fsp[:, -1:]], axis=1)  # h+1 clamped
    sdh = np.concatenate([sd[:, 1:], sd[:, -1:]], axis=1)   # d+1,h+1 clamped
    v4 = np.empty((D, H, W, 4, 4), np.float32)
    v4[:, :, :, 0] = fsp
    v4[:, :, :, 1] = sd
    v4[:, :, :, 2] = sh
    v4[:, :, :, 3] = sdh
    del fsp, sd, sh, sdh
    v4 = v4.reshape(NVOX, ROW)
    return np.concatenate([v4, np.zeros((2, ROW), np.float32)], axis=0)


def _pack_points(pts):
    """pts [NPC,3] -> [P, 3*COLS] with point n=j*P+p at [p, 3j:3j+3]."""
    arr = np.zeros((NPAD, 3), np.float32)
    arr[:NPC] = pts
    return np.ascontiguousarray(arr.reshape(COLS, P, 3).transpose(1, 0, 2)).reshape(P, 3 * COLS)


def _unpack_points(xout):
    return xout.reshape(P, COLS, 3).transpose(1, 0, 2).reshape(NPAD, 3)[:NPC]


# ------------------------------------------------------------- numpy simulator

def _simulate_core(v4, xin):
    """Bit-faithful numpy model of the device program (for logic validation)."""
    x = xin.reshape(P, COLS, 3).copy()
    volf = v4.ravel()
    for _s in range(NUM_STEPS):
        p = np.clip(x, 0.0, 191.0)
        fl = np.floor(p)
        f = p - fl
        v = (fl[..., 0] * 36864.0 + fl[..., 1] * 192.0 + fl[..., 2]).astype(np.int64)
        # gather 32 floats per point
        idx = v[..., None] * ROW + np.arange(GELEM)
        G = volf[idx]                                # [P, COLS, 32]
        g0, g1 = G[..., :16], G[..., 16:]
        A = g0 + f[..., 2:3] * (g1 - g0)             # w-lerp  -> 16
        Bv = A[..., :8] + f[..., 1:2] * (A[..., 8:] - A[..., :8])   # h-lerp -> 8
        r = Bv[..., :4] + f[..., 0:1] * (Bv[..., 4:] - Bv[..., :4])  # d-lerp -> 4
        x = x + r[..., :3]
    return x.reshape(P, COLS * 3)


# ------------------------------------------------------------- device program

def _build_program():
    ablate = os.environ.get("DIFFEO_ABLATE", "")
    nc = bass.Bass()
    v4 = nc.dram_tensor("v4", [NVOX + 2, ROW], F32, kind="ExternalInput")
    xin = nc.dram_tensor("xin", [P, 3 * COLS], F32, kind="ExternalInput")
    voff0 = None
    if ablate == "hostoff":
        voff0 = nc.dram_tensor("voff0", [P, COLS], I32, kind="ExternalInput")
    xout = nc.dram_tensor("xout", [P, 3 * COLS], F32, kind="ExternalOutput")

    ngroups = len(GROUP_SIZES)
    gstart = np.cumsum([0] + GROUP_SIZES).tolist()
    maxg = max(GROUP_SIZES)

    # semaphore count bookkeeping (python-side, static)
    dmag_cnt = [0, 0, 0, 0]
    dve_cnt = [0]

    from contextlib import ExitStack

    with ExitStack() as ctx:
        sb = lambda name, shape, dt: ctx.enter_context(nc.sbuf_tensor(name, shape, dt))
        t_x = sb("t_x", [P, 3 * COLS], F32)
        t_f = sb("t_f", [P, 3 * COLS], F32)
        t_voff = sb("t_voff", [P, COLS], I32)
        t_g = sb("t_g", [P, ngroups * maxg * GELEM], F32)
        t_p = sb("t_p", [P, 3 * maxg], F32)
        t_fl = sb("t_fl", [P, 3 * maxg], F32)
        t_m = sb("t_m", [P, 3 * maxg], F32)
        t_wi = sb("t_wi", [P, 3 * maxg], I32)
        t_a = sb("t_a", [P, maxg], F32)
        t_b = sb("t_b", [P, maxg], F32)
        t_v = sb("t_v", [P, maxg], F32)
        t_D = sb("t_D", [P, 16 * maxg], F32)
        t_A = sb("t_A", [P, 16 * maxg], F32)
        t_D2 = sb("t_D2", [P, 8 * maxg], F32)
        t_B = sb("t_B", [P, 8 * maxg], F32)
        t_D3 = sb("t_D3", [P, 4 * maxg], F32)
        t_R = sb("t_R", [P, 4 * maxg], F32)
        dmag = [ctx.enter_context(nc.semaphore(f"dmag{i}")) for i in range(4)]
        dve_sem = ctx.enter_context(nc.semaphore("dve_sem"))
        block = ctx.enter_context(nc.Block())

        # static schedules for cross-engine wait values
        # addr_done[s][g]: dve_sem value after addr phase (s,g)
        # lerp_done[s][g]: dve_sem value after lerp phase (s,g)
        # gath_done[s][g]: (q, dmag value) after all gathers of (s,g)
        addr_done = [[0] * ngroups for _ in range(NUM_STEPS)]
        lerp_done = [[0] * ngroups for _ in range(NUM_STEPS)]
        gath_done = [[0] * ngroups for _ in range(NUM_STEPS)]
        # DVE program order: addr(0,*) prologue, then per step and group:
        # lerp(s,g) followed immediately by addr(s+1,g), so the Pool can start
        # step s+1 group 0 gathers while still draining step s group 1.
        c = 0
        for g in range(ngroups):
            c += 1
            addr_done[0][g] = c
        for s in range(NUM_STEPS):
            for g in range(ngroups):
                c += 1
                lerp_done[s][g] = c
                if s + 1 < NUM_STEPS:
                    c += 1
                    addr_done[s + 1][g] = c
        dve_total = c
        # preload on dmag0
        pre = [32 if ablate == "hostoff" else 16, 0, 0, 0]
        cnt = list(pre)
        for s in range(NUM_STEPS):
            q = s % 4
            for g in range(ngroups):
                cnt[q] += 16 * GROUP_SIZES[g]
                gath_done[s][g] = (q, cnt[q])

        @block.gpsimd
        def _(gpsimd):
            gpsimd.dma_start(t_x[:], xin[:]).then_inc(dmag[0], 16)
            if ablate == "hostoff":
                gpsimd.dma_start(t_voff[:], voff0[:]).then_inc(dmag[0], 16)
                gpsimd.wait_ge(dmag[0], 32)
            for s in range(NUM_STEPS):
                q = s % 4
                for g in range(ngroups):
                    gpsimd.wait_ge(dve_sem, addr_done[s][g])
                    gs, gn = gstart[g], GROUP_SIZES[g]
                    for jj in range(gn):
                        col = gs + jj
                        if ablate == "nogather":
                            gpsimd.dma_start(
                                out=t_g[:, (g * maxg + jj) * GELEM:(g * maxg + jj + 1) * GELEM],
                                in_=bass.AP(v4[:].tensor, 0, [[GELEM, P], [1, GELEM]]),
                            ).then_inc(dmag[q], 16)
                        else:
                            gpsimd.indirect_dma_start(
                                out=t_g[:, (g * maxg + jj) * GELEM:(g * maxg + jj + 1) * GELEM],
                                out_offset=None,
                                in_=v4[:],
                                in_offset=bass.IndirectOffsetOnAxis(
                                    ap=t_voff[:, col:col + 1], axis=0
                                ),
                            ).then_inc(dmag[q], 16)
            gpsimd.wait_ge(dve_sem, dve_total)
            gpsimd.dma_start(xout[:], t_x[:]).then_inc(dmag[1], 16)

        @block.vector
        def _(vector):
            tt = vector.tensor_tensor
            ts = vector.tensor_scalar

            def addr_phase(g):
                if True:
                    gs, gn = gstart[g], GROUP_SIZES[g]
                    w3 = 3 * gn
                    xg = t_x[:, 3 * gs:3 * gs + w3]
                    fg = t_f[:, 3 * gs:3 * gs + w3]
                    pg = t_p[:, :w3]
                    flg = t_fl[:, :w3]
                    mg = t_m[:, :w3]
                    wig = t_wi[:, :w3]
                    # p = clip(x, 0, 191)
                    ts(out=pg, in0=xg, scalar1=191.0, scalar2=None, op0=Alu.min)
                    ts(out=pg, in0=pg, scalar1=0.0, scalar2=None, op0=Alu.max)
                    # floor via cast + correction
                    vector.tensor_copy(out=wig, in_=pg)
                    vector.tensor_copy(out=flg, in_=wig)
                    tt(out=mg, in0=flg, in1=pg, op=Alu.is_gt)
                    tt(out=flg, in0=flg, in1=mg, op=Alu.subtract)
                    tt(out=fg, in0=pg, in1=flg, op=Alu.subtract)
                    # voxel index v = d*36864 + h*192 + w
                    fl3 = flg.rearrange("p (t c) -> p t c", c=3)
                    ag = t_a[:, :gn]
                    bg = t_b[:, :gn]
                    vg = t_v[:, :gn]
                    ts(out=ag, in0=fl3[:, :, 0], scalar1=36864.0, scalar2=None, op0=Alu.mult)
                    ts(out=bg, in0=fl3[:, :, 1], scalar1=192.0, scalar2=None, op0=Alu.mult)
                    tt(out=vg, in0=ag, in1=bg, op=Alu.add)
                    tt(out=vg, in0=vg, in1=fl3[:, :, 2], op=Alu.add)
                    voff_dst = t_wi[:, :gn].bitcast(I32) if ablate == "hostoff" else t_voff[:, gs:gs + gn]
                    vector.tensor_copy(out=voff_dst, in_=vg).then_inc(dve_sem, 1)

            def lerp_phase(s, g):
                gs, gn = gstart[g], GROUP_SIZES[g]
                qq, val = gath_done[s][g]
                vector.wait_ge(dmag[qq], val)
                if ablate == "nolerp":
                    vector.tensor_copy(out=t_R[:, :1], in_=t_B[:, :1]).then_inc(dve_sem, 1)
                    return
                if True:
                    gb = t_g[:, g * maxg * GELEM:(g * maxg + gn) * GELEM]
                    g0 = bass.AP(gb.tensor, gb.offset, [[gb.ap[0][0], P], [GELEM, gn], [1, 16]])
                    g1 = bass.AP(gb.tensor, gb.offset + 16, [[gb.ap[0][0], P], [GELEM, gn], [1, 16]])
                    fg = t_f[:, 3 * gs:3 * gs + 3 * gn]
                    fw = bass.AP(fg.tensor, fg.offset + 2, [[fg.ap[0][0], P], [3, gn], [0, 16]])
                    fh = bass.AP(fg.tensor, fg.offset + 1, [[fg.ap[0][0], P], [3, gn], [0, 8]])
                    fd = bass.AP(fg.tensor, fg.offset + 0, [[fg.ap[0][0], P], [3, gn], [0, 4]])
                    Dg = t_D[:, :16 * gn].rearrange("p (t e) -> p t e", e=16)
                    Ag = t_A[:, :16 * gn].rearrange("p (t e) -> p t e", e=16)
                    D2g = t_D2[:, :8 * gn].rearrange("p (t e) -> p t e", e=8)
                    Bg = t_B[:, :8 * gn].rearrange("p (t e) -> p t e", e=8)
                    D3g = t_D3[:, :4 * gn].rearrange("p (t e) -> p t e", e=4)
                    Rg = t_R[:, :4 * gn].rearrange("p (t e) -> p t e", e=4)
                    A0 = t_A[:, :16 * gn].rearrange("p (t k e) -> p t k e", k=2, e=8)
                    B0 = t_B[:, :8 * gn].rearrange("p (t k e) -> p t k e", k=2, e=4)
                    # w-lerp
                    tt(out=Dg, in0=g1, in1=g0, op=Alu.subtract)
                    if ablate == "lerp0":
                        vector.tensor_copy(out=t_R[:, :1], in_=t_B[:, :1]).then_inc(dve_sem, 1)
                        return
                    tt(out=Dg, in0=Dg, in1=fw, op=Alu.mult)
                    tt(out=Ag, in0=Dg, in1=g0, op=Alu.add)
                    if ablate == "lerp1":
                        vector.tensor_copy(out=t_R[:, :1], in_=t_B[:, :1]).then_inc(dve_sem, 1)
                        return
                    # h-lerp
                    tt(out=D2g, in0=A0[:, :, 1], in1=A0[:, :, 0], op=Alu.subtract)
                    tt(out=D2g, in0=D2g, in1=fh, op=Alu.mult)
                    tt(out=Bg, in0=D2g, in1=A0[:, :, 0], op=Alu.add)
                    if ablate == "lerp2":
                        vector.tensor_copy(out=t_R[:, :1], in_=t_B[:, :1]).then_inc(dve_sem, 1)
                        return
                    # d-lerp
                    tt(out=D3g, in0=B0[:, :, 1], in1=B0[:, :, 0], op=Alu.subtract)
                    tt(out=D3g, in0=D3g, in1=fd, op=Alu.mult)
                    tt(out=Rg, in0=D3g, in1=B0[:, :, 0], op=Alu.add)
                    if ablate == "lerp3":
                        vector.tensor_copy(out=t_R[:, :1], in_=t_B[:, :1]).then_inc(dve_sem, 1)
                        return
                    # x += r (first 3 of 4 channels)
                    xg3 = t_x[:, 3 * gs:3 * gs + 3 * gn].rearrange("p (t c) -> p t c", c=3)
                    r3 = bass.AP(t_R[:].tensor, t_R[:].offset, [[t_R[:].ap[0][0], P], [4, gn], [1, 3]])
                    tt(out=xg3, in0=xg3, in1=r3, op=Alu.add).then_inc(dve_sem, 1)

            vector.wait_ge(dmag[0], 16)
            for g in range(ngroups):
                addr_phase(g)
            for s in range(NUM_STEPS):
                for g in range(ngroups):
                    lerp_phase(s, g)
                    if s + 1 < NUM_STEPS:
                        addr_phase(g)

    return nc


_cached = {}


def _get_program():
    if "nc" not in _cached:
        _cached["nc"] = _build_program()
    return _cached["nc"]


LAST_EXEC_TIME_NS = None


def kernel(verts, affine, flow_field):
    global LAST_EXEC_TIME_NS
    verts = np.asarray(verts, np.float32)
    affine = np.asarray(affine, np.float32)
    flow_field = np.asarray(flow_field, np.float32)

    # host: forward affine
    pred0 = np.empty((B, N, 3), np.float32)
    for b in range(B):
        A = affine[b]
        pred0[b] = verts[b] @ A[:3, :3].T + A[:3, 3]

    v4s = [_build_v4(flow_field[b]) for b in range(B)]

    in_maps = []
    per_batch = NCORES // B
    for c in range(NCORES):
        b = c // per_batch
        q = c % per_batch
        pts = pred0[b, q * NPC:(q + 1) * NPC]
        in_maps.append({"v4": v4s[b], "xin": _pack_points(pts)})

    if os.environ.get("DIFFEO_SIM") == "1":
        results = [{"xout": _simulate_core(m["v4"], m["xin"])} for m in in_maps]
    else:
        from concourse.bass_utils import run_bass_kernel_spmd

        nc = _get_program()
        trace = os.environ.get("DIFFEO_TRACE") == "1"
        res = run_bass_kernel_spmd(nc, in_maps, core_ids=list(range(NCORES)), trace=trace)
        LAST_EXEC_TIME_NS = res.exec_time_ns
        results = res.results

    aux = np.empty((B, N, 3), np.float32)
    for c in range(NCORES):
        b = c // per_batch
        q = c % per_batch
        aux[b, q * NPC:(q + 1) * NPC] = _unpack_points(results[c]["xout"])

    flow_int = np.transpose(aux - pred0, (0, 2, 1))[..., None, None]
    pred_out = np.empty((B, N, 3), np.float32)
    for b in range(B):
        iA = np.linalg.inv(affine[b])
        pred_out[b] = aux[b] @ iA[:3, :3].T + iA[:3, 3]
    return pred_out, flow_int
